# revision 26
# baseline (speedup 1.0000x reference)
import numpy as np

B = 128
FEAT = 64
LATENT = 512
OUT_F = 6144  # NUM_POINTS * 3
OUT_C = OUT_F // 8  # 768 output cols per core (w3 column-sharded)
EPS = 1e-5
N_CORES = 8
SEGS_PER_CORE = 16
S_PAD = 8192
W = 512            # wave width (cols per slot per wave)
NW = S_PAD // W    # 16 waves
F16MIN = np.float16(-65504.0)

_CACHE = {}


def build_nc():
    from concourse import bass, bacc, tile

    mybir = bass.mybir
    f32 = mybir.dt.float32
    f16 = mybir.dt.float16
    bf16 = mybir.dt.bfloat16
    AF = mybir.ActivationFunctionType
    ALU = mybir.AluOpType
    X = mybir.AxisListType.X

    nc = bacc.Bacc("TRN2", num_devices=N_CORES)
    # wave-major feat layout: 16 waves of [8 slots x 512]
    xt_d = nc.declare_dram_parameter("xt", [128, NW, 8 * W], f16, isOutput=False)
    # proj with LN affine folded in + bias as a 65th contraction row
    wpe_d = nc.declare_dram_parameter("wpe", [64, LATENT], bf16, isOutput=False)
    bp_d = nc.declare_dram_parameter("bpr", [1, LATENT], f32, isOutput=False)
    w1_d = nc.declare_dram_parameter("w1p", [128, 1024], bf16, isOutput=False)
    b1_d = nc.declare_dram_parameter("b1r", [1, 256], f32, isOutput=False)
    w2_d = nc.declare_dram_parameter("w2p", [128, 1024], bf16, isOutput=False)
    b2_d = nc.declare_dram_parameter("b2r", [1, 512], f32, isOutput=False)
    # per-core w3 column shard: [k-chunk partition, 4 chunks, 768 cols]
    w3_d = nc.declare_dram_parameter("w3s", [128, 4, OUT_C], bf16, isOutput=False)
    idn_d = nc.declare_dram_parameter("idn", [128, 128], f32, isOutput=False)
    # out rows = global segment ids (post all-gather), cols = this core's slice
    out_d = nc.declare_dram_parameter("out", [128, OUT_C], f16, isOutput=True)

    with tile.TileContext(nc) as tc:
        with (
            tc.tile_pool(name="wpool", bufs=1) as wpool,
            tc.tile_pool(name="fpool", bufs=5) as fpool,
            tc.tile_pool(name="spool", bufs=1) as spool,
            tc.tile_pool(name="dram", bufs=1, space="DRAM") as dram,
            tc.tile_pool(name="ps_s", bufs=1, space=bass.MemorySpace.PSUM) as ps_s,
            tc.tile_pool(name="ps_m", bufs=2, space=bass.MemorySpace.PSUM) as ps_m,
            tc.tile_pool(name="ps_o", bufs=2, space=bass.MemorySpace.PSUM) as ps_o,
        ):
            acc = wpool.tile([128, 8, W], f16)
            wpe = wpool.tile([64, LATENT], bf16)
            bpr = wpool.tile([1, LATENT], f32)
            w1 = wpool.tile([128, 1024], bf16)
            b1r = wpool.tile([1, 256], f32)
            w2 = wpool.tile([128, 1024], bf16)
            b2r = wpool.tile([1, 512], f32)
            w3sb = wpool.tile([128, 4, OUT_C], bf16)
            idn = wpool.tile([128, 128], f32)
            eps128 = wpool.tile([128, 1], f32)
            ones1 = wpool.tile([1, 128], f32)
            scr = wpool.tile([128, 1], f32)
            zTe = wpool.tile([64, 128], bf16)

            # wave 0 lands directly in the accumulator (no memset / no fold)
            nc.sync.dma_start(acc[:, :, :], xt_d[:, 0, :])

            # small weights + the w3 shard on the scalar HWDGE queue; the w3
            # shard is only 0.79MB so it no longer fights the feat stream.
            for t, d in (
                (wpe, wpe_d), (bpr, bp_d), (w1, w1_d), (b1r, b1_d),
                (w2, w2_d), (b2r, b2_d), (idn, idn_d),
            ):
                nc.scalar.dma_start(t[:], d[:])
            nc.scalar.dma_start(w3sb[:], w3_d[:])

            nc.vector.memset(eps128[:], EPS)
            nc.vector.memset(ones1[:], 1.0)
            # hoist the Sqrt activation-table load out of the tail
            nc.scalar.activation(scr[:], eps128[:], AF.Sqrt)

            # collective bounce buffers
            cc_in = dram.tile([16, 64], f32)
            cc_out = dram.tile([128, 64], f32)

            val_h = spool.tile([128, 8], f16)
            val32 = spool.tile([128, 8], f32)
            tpsb = spool.tile([8, 64], f32)
            zloc = spool.tile([16, 64], f32)
            zall = spool.tile([128, 64], f32)
            ssum = spool.tile([128, 1], f32)
            nmu = spool.tile([128, 1], f32)
            vsum = spool.tile([128, 1], f32)
            std = spool.tile([128, 1], f32)
            rstd = spool.tile([128, 1], f32)
            zc = spool.tile([128, 64], f32)
            sq = spool.tile([128, 64], f32)
            zn = spool.tile([128, 64], f32)
            latT = spool.tile([128, 4, 128], bf16)
            h1T = spool.tile([128, 2, 128], bf16)
            h2T = spool.tile([128, 4, 128], bf16)
            out_sb = spool.tile([128, OUT_C], f16)

            # --- streaming waves: one DMA + one [128, 8*W] DVE max fold per
            # wave. f16 TT runs in 2x mode (2 elem/cyc/lane). ---
            for ci in range(1, NW):
                ft = fpool.tile([128, 8, W], f16, name="ft")
                nc.sync.dma_start(ft[:], xt_d[:, ci, :])
                nc.vector.tensor_tensor(acc[:], acc[:], ft[:], op=ALU.max)
                if ci == 10:
                    # PE warm-up burst gated on stream progress so HAM is
                    # warm for the tail matmuls
                    gate = ps_s.tile([8, 128], f32, name="gate")
                    nc.tensor.matmul(
                        gate[0:2, 0:1], ft[:, 0, 0:2], ft[:, 0, 0:1],
                        start=True, stop=True,
                    )
                    wps = ps_o.tile([128, 512], f32, name="pso")
                    for _ in range(12):
                        nc.tensor.matmul(
                            wps[0:16, :], w1[:, 0:16], w1[:, 0:512],
                            start=True, stop=True,
                        )

            # --- final tree-fold + per-slot reduce -> val_h [128, 8] ---
            nc.vector.tensor_tensor(
                acc[:, :, 0:256], acc[:, :, 0:256], acc[:, :, 256:512], op=ALU.max
            )
            nc.vector.tensor_tensor(
                acc[:, :, 0:128], acc[:, :, 0:128], acc[:, :, 128:256], op=ALU.max
            )
            nc.vector.tensor_tensor(
                acc[:, :, 0:64], acc[:, :, 0:64], acc[:, :, 64:128], op=ALU.max
            )
            nc.vector.reduce_max(val_h[:].rearrange("p (a b) -> p a b", b=1),
                                 acc[:, :, 0:64], axis=X)
            nc.vector.tensor_copy(val32[:], val_h[:])

            import os as _os
            _stop = int(_os.environ.get("DBG_STOP", "99"))

            # --- local pooled [128(g*64+f), 8(t)] -> [16 seg, 64 feat] via
            # PE transpose + one partition-moving SBUF DMA, then all-gather
            # so every core sees all 128 segments ---
            tp_ps = ps_s.tile([8, 128], f32, name="tp")
            nc.tensor.transpose(tp_ps[:], val32[:], idn[:])
            nc.vector.tensor_copy(zloc[0:8, :], tp_ps[0:8, 0:64])
            nc.scalar.copy(tpsb[:], tp_ps[0:8, 64:128])
            nc.sync.dma_start(zloc[8:16, :], tpsb[:])
            nc.sync.dma_start(cc_in[:], zloc[:])
            import os as _os
            if _os.environ.get("DBG_NO_CC"):
                nc.vector.memset(zall[:], 1.0)
                nc.sync.dma_start(zall[0:16, :], cc_in[:])
            else:
                nc.gpsimd.collective_compute(
                    "AllGather",
                    ALU.bypass,
                    replica_groups=[list(range(N_CORES))],
                    ins=[cc_in[:].opt()],
                    outs=[cc_out[:].opt()],
                )
                nc.sync.dma_start(zall[:], cc_out[:])

            # --- LayerNorm on [128 seg, 64 feat]: free-axis reduces on DVE ---
            nc.vector.reduce_sum(ssum[:], zall[:], axis=X)
            nc.vector.tensor_scalar(nmu[:], ssum[:], -1.0 / FEAT, None, op0=ALU.mult)
            nc.vector.tensor_scalar(zc[:], zall[:], nmu[:], None, op0=ALU.add)
            nc.vector.tensor_tensor(sq[:], zc[:], zc[:], op=ALU.mult)
            nc.vector.reduce_sum(vsum[:], sq[:], axis=X)
            nc.scalar.activation(std[:], vsum[:], AF.Sqrt,
                                 bias=eps128[:], scale=1.0 / FEAT)
            nc.vector.reciprocal(rstd[:], std[:])
            nc.vector.tensor_scalar(zn[:], zc[:], rstd[:], None, op0=ALU.mult)

            # --- transpose zn -> zTe[0:64] (seg on cols); row 64 is ones so
            # the proj bias rides the contraction ---
            znT_ps = ps_s.tile([64, 128], f32, name="znT")
            nc.tensor.transpose(znT_ps[:], zn[:], idn[:])
            nc.vector.tensor_copy(zTe[:], znT_ps[:])

            # --- latT[l, s] (4 slices of 128): wpe^T @ zTe ---
            for m in range(4):
                ps = ps_m.tile([128, 128], f32)
                nc.tensor.matmul(
                    ps[:], wpe[:, 128 * m : 128 * (m + 1)], zTe[:],
                    start=True, stop=False,
                )
                nc.tensor.matmul(
                    ps[:], bpr[:, 128 * m : 128 * (m + 1)], ones1[:],
                    start=False, stop=True,
                )
                eng = nc.vector if m % 2 == 0 else nc.scalar
                if m % 2 == 0:
                    nc.vector.tensor_copy(latT[:, m, :], ps[:])
                else:
                    nc.scalar.copy(latT[:, m, :], ps[:])

            # --- h1T = relu(w1^T @ latT + b1) ---
            for n in range(2):
                ps = ps_m.tile([128, 128], f32)
                for k in range(4):
                    nc.tensor.matmul(
                        ps[:],
                        w1[:, (k * 2 + n) * 128 : (k * 2 + n + 1) * 128],
                        latT[:, k, :],
                        start=(k == 0), stop=False,
                    )
                nc.tensor.matmul(
                    ps[:], b1r[:, 128 * n : 128 * (n + 1)], ones1[:],
                    start=False, stop=True,
                )
                nc.vector.tensor_scalar(
                    h1T[:, n, :], ps[:], 0.0, None, op0=ALU.max
                )

            # --- h2T = relu(w2^T @ h1T + b2) ---
            for n in range(4):
                ps = ps_m.tile([128, 128], f32)
                for k in range(2):
                    nc.tensor.matmul(
                        ps[:],
                        w2[:, (k * 4 + n) * 128 : (k * 4 + n + 1) * 128],
                        h1T[:, k, :],
                        start=(k == 0), stop=False,
                    )
                nc.tensor.matmul(
                    ps[:], b2r[:, 128 * n : 128 * (n + 1)], ones1[:],
                    start=False, stop=True,
                )
                if n % 2 == 0:
                    nc.vector.tensor_scalar(
                        h2T[:, n, :], ps[:], 0.0, None, op0=ALU.max
                    )
                else:
                    nc.scalar.activation(h2T[:, n, :], ps[:], AF.Relu)

            if _stop <= 7:
                nc.vector.memset(out_sb[:], 0.0)
                nc.vector.tensor_copy(out_sb[:, 0:128], h2T[:, 0, :])
                nc.sync.dma_start(out_d[:], out_sb[:])
                _emit_gemm = False
            else:
                _emit_gemm = True
            # --- out slice = h2^T... stationary h2T chunks [128, 128 segs],
            # moving = this core's w3 shard -> psum [128 segs, 768] ---
            if _emit_gemm:
                pso1 = ps_o.tile([128, 512], f32, name="pso")
                for k in range(4):
                    nc.tensor.matmul(
                        pso1[:], h2T[:, k, :], w3sb[:, k, 0:512],
                        start=(k == 0), stop=(k == 3),
                    )
                pso2 = ps_o.tile([128, 512], f32, name="pso")
                for k in range(4):
                    nc.tensor.matmul(
                        pso2[:, 0:256], h2T[:, k, :], w3sb[:, k, 512:768],
                        start=(k == 0), stop=(k == 3),
                    )
                nc.vector.tensor_copy(out_sb[:, 0:512], pso1[:])
                nc.sync.dma_start(out_d[:, 0:512], out_sb[:, 0:512])
                nc.scalar.copy(out_sb[:, 512:768], pso2[:, 0:256])
                nc.sync.dma_start(out_d[:, 512:768], out_sb[:, 512:768])

    nc.finalize()
    return nc


def _bf16(a):
    import ml_dtypes

    return np.ascontiguousarray(a.astype(ml_dtypes.bfloat16))


def pack_weights(ln_g, ln_b, proj_w, proj_b, w1, b1, w2, b2, w3, b3):
    c = np.ascontiguousarray
    wp = (ln_g[:, None] * proj_w).astype(np.float32)  # [64, 512]
    bpv = (ln_b.astype(np.float64) @ proj_w.astype(np.float64)).astype(np.float32) + proj_b
    wdict = {
        "wpe": _bf16(wp),
        "bpr": c(bpv[None, :]),
        "w1p": _bf16(
            w1.reshape(4, 128, 2, 128).transpose(1, 0, 2, 3).reshape(128, 1024)
        ),
        "b1r": c(b1[None, :]),
        "w2p": _bf16(
            w2.reshape(2, 128, 4, 128).transpose(1, 0, 2, 3).reshape(128, 1024)
        ),
        "b2r": c(b2[None, :]),
        "idn": c(np.eye(128, dtype=np.float32)),
    }
    # per-core w3 column shards: [p, k, j] = w3[128k+p, OUT_C*c + j]
    w3r = w3.reshape(4, 128, 8, OUT_C)
    w3s = [_bf16(np.ascontiguousarray(w3r[:, :, cc, :].transpose(1, 0, 2)))
           for cc in range(N_CORES)]
    return wdict, w3s


def pack_feat_core(feat16, feat32, bounds, c):
    xt = np.full((128, 8, S_PAD), F16MIN, np.float16)
    for sl in range(SEGS_PER_CORE):
        seg = c * SEGS_PER_CORE + sl
        a, b = bounds[seg], bounds[seg + 1]
        L = b - a
        if L > S_PAD:
            blk = np.concatenate(
                [
                    feat16[a : a + S_PAD - 1],
                    feat32[a + S_PAD - 1 : b].max(0, keepdims=True).astype(np.float16),
                ],
                0,
            )
            L = S_PAD
        else:
            blk = feat16[a:b]
        g, t = divmod(sl, 8)
        if L > 0:
            xt[g * 64 : (g + 1) * 64, t, :L] = blk.T
    # wave-major reorder: [128, slot, wave, W] -> [128, wave, slot, W]
    return np.ascontiguousarray(
        xt.reshape(128, 8, NW, W).swapaxes(1, 2)
    ).reshape(128, NW, 8 * W)


def make_in_maps(inputs):
    feat32 = np.asarray(inputs["feat"], dtype=np.float32)
    feat16 = feat32.astype(np.float16)
    batch = np.asarray(inputs["batch"])
    wdict, w3s = pack_weights(
        *(np.asarray(inputs[k], dtype=np.float32) for k in
          ("ln_g", "ln_b", "proj_w", "proj_b", "w1", "b1", "w2", "b2", "w3", "b3"))
    )
    bounds = np.searchsorted(batch, np.arange(B + 1))
    return [
        {"xt": pack_feat_core(feat16, feat32, bounds, c), "w3s": w3s[c], **wdict}
        for c in range(N_CORES)
    ]


def kernel(**inputs):
    from concourse.bass_utils import run_bass_kernel_spmd

    if "nc" not in _CACHE:
        _CACHE["nc"] = build_nc()
    nc = _CACHE["nc"]

    in_maps = make_in_maps(inputs)
    res = run_bass_kernel_spmd(nc, in_maps, list(range(N_CORES)))

    out = np.empty((B, OUT_F), np.float32)
    for c in range(N_CORES):
        r = np.asarray(res.results[c]["out"], dtype=np.float32)  # [128, 768]
        out[:, OUT_C * c : OUT_C * (c + 1)] = r
    out += np.asarray(inputs["b3"], dtype=np.float32)[None, :]
    return out.reshape(B, 2048, 3)
